# revision 8
# baseline (speedup 1.0000x reference)
"""FSUMGU cell on 8 Trainium2 NeuronCores.

Math (per reference):
    zf = [hx, x] @ w_f.T + b_f
    fg = (zf + 1) / 2
    fgx = fg * hx
    ng = [fgx, x] @ w_n.T + b_n
    hy = (1 - fg) * ng + fgx

Sharding: 2 batch-halves (r) x 4 hidden-quarters (c); core id = r*4 + c.
Each core computes hy[r-half, c-quarter]. The only cross-core dependency
is ng's contraction over the full hidden dim of fgx, satisfied with one
AllGather of bf16 fgx^T over each 4-core row group.

Layouts on core: activations/weights are PE-transposed into [k, *] SBUF
tiles (bf16) so every matmul is out[b,h] += catT[k,b].T @ wT[k,h] with
fp32 PSUM accumulation. fp32->bf16 happens inside SWDGE cast-DMAs.
"""
import sys

sys.path.insert(0, "/opt/trn_rl_repo")

import numpy as np
import concourse.bass as bass
import concourse.tile as tile
from concourse import bacc, mybir, masks
from concourse.bass_utils import run_bass_kernel_spmd

F32 = mybir.dt.float32
BF16 = mybir.dt.bfloat16

B, H, I = 2048, 2048, 2048
R, C = 2, 4
BL = B // R            # 1024 rows of batch per core
HC = H // C            # 512 output features per core
NB = BL // 128         # 8 batch tiles
NKH = H // 128         # 16 k-tiles in the hx / fgx part
NKI = I // 128         # 16 k-tiles in the input part
NK = NKH + NKI         # 32 k-tiles total contraction
NHT = HC // 128        # 4 h-tiles per core slice

_NC_CACHE = None


def build():
    nc = bacc.Bacc(None, target_bir_lowering=False, debug=False)
    d_inp = nc.dram_tensor("inp", [BL, I], F32, kind="ExternalInput").ap()
    d_hx = nc.dram_tensor("hx", [BL, H], F32, kind="ExternalInput").ap()
    d_hxc = nc.dram_tensor("hxc", [BL, HC], F32, kind="ExternalInput").ap()
    d_wf = nc.dram_tensor("wf", [HC, H + I], F32, kind="ExternalInput").ap()
    d_wn = nc.dram_tensor("wn", [HC, H + I], F32, kind="ExternalInput").ap()
    d_bf = nc.dram_tensor("bf", [1, HC], F32, kind="ExternalInput").ap()
    d_bn = nc.dram_tensor("bn", [1, HC], F32, kind="ExternalInput").ap()
    d_hy = nc.dram_tensor("hy", [BL, HC], F32, kind="ExternalOutput").ap()

    with tile.TileContext(nc) as tc:
        with (
            tc.tile_pool(name="const", bufs=1) as const,
            tc.tile_pool(name="wT", bufs=1) as wT_pool,          # wfT then wnT (time-shared)
            tc.tile_pool(name="big", bufs=1) as big_pool,        # hxT then gather (time-shared) + inputT
            tc.tile_pool(name="persist", bufs=1) as persist,
            tc.tile_pool(name="aload", bufs=2) as aload,
            tc.tile_pool(name="wload", bufs=3) as wload,
            tc.tile_pool(name="scr", bufs=2) as scr,
            tc.tile_pool(name="fgtr", bufs=2) as fgtr,
            tc.tile_pool(name="outp", bufs=2) as outp,
            tc.tile_pool(name="dram", bufs=1, space="DRAM") as dram,
            tc.tile_pool(name="ps_acc", bufs=3, space="PSUM") as ps_acc,
            tc.tile_pool(name="ps_tp", bufs=3, space="PSUM") as ps_tp,
            tc.tile_pool(name="ps_b", bufs=1, space="PSUM") as ps_b,
        ):
            ident = const.tile([128, 128], BF16, tag="ident")
            masks.make_identity(nc, ident[:])
            ones = const.tile([1, 128], BF16, tag="ones")
            nc.vector.memset(ones[:], 1.0)

            # ---- bias prep: bfp=(b_f+1)/2, bfm=1-bfp, bn; broadcast to 128 partitions
            bf_row = const.tile([1, HC], F32, tag="bfrow")
            bn_row = const.tile([1, HC], F32, tag="bnrow")
            nc.sync.dma_start(bf_row[:], d_bf[:])
            nc.sync.dma_start(bn_row[:], d_bn[:])
            bfp_row = const.tile([1, HC], F32, tag="bfprow")
            bfm_row = const.tile([1, HC], F32, tag="bfmrow")
            # bfp = 0.5*b_f + 0.5 ; bfm = 1 - bfp
            nc.vector.tensor_scalar(bfp_row[:], bf_row[:], 0.5, 0.5,
                                    mybir.AluOpType.mult, mybir.AluOpType.add)
            nc.vector.tensor_scalar(bfm_row[:], bfp_row[:], -1.0, 1.0,
                                    mybir.AluOpType.mult, mybir.AluOpType.add)
            bias_bc = const.tile([128, 3, HC], BF16, tag="biasbc")
            for bi, row in enumerate((bfp_row, bfm_row, bn_row)):
                row16 = const.tile([1, HC], BF16, tag=f"row16_{bi}")
                nc.vector.tensor_copy(row16[:], row[:])
                pb = ps_b.tile([128, HC], F32, tag="pbias")
                nc.tensor.matmul(pb[:], ones[:], row16[:], start=True, stop=True)
                nc.vector.tensor_copy(bias_bc[:, bi, :], pb[:])
            bfp_bc = bias_bc[:, 0, :]
            bfm_bc = bias_bc[:, 1, :]
            bn_bc = bias_bc[:, 2, :]

            # ---- persistent transposed tensors
            hxT = big_pool.tile([128, NKH, BL], BF16, tag="big_hx")     # hx^T  (phase 1)
            inputT = big_pool.tile([128, NKI, BL], BF16, tag="big_inp")  # input^T (both phases)
            wfT = wT_pool.tile([128, NK, HC], BF16, tag="wT")
            fg_hxT = persist.tile([128, NHT, BL], BF16, tag="fghxT")
            omfgN = persist.tile([128, NB, HC], BF16, tag="omfg")
            fghxN = persist.tile([128, NB, HC], BF16, tag="fghx")
            hxcN = persist.tile([128, NB, HC], F32, tag="hxc")

            # DRAM bounce buffers for the collective
            cc_in = dram.tile([HC, BL], BF16)
            cc_out = dram.tile([C, HC, BL], BF16)

            def transpose_chunk(src_b16, n_kt, dst, dst_ti0, dst_col, eng_sel):
                """PE-transpose n_kt [128,128] slices of src into dst[:, ti0+j, col:col+128]."""
                for g in range(0, n_kt, 4):
                    tp = ps_tp.tile([128, 512], BF16, tag="tp")
                    for j in range(g, min(g + 4, n_kt)):
                        nc.tensor.matmul(
                            tp[:, (j - g) * 128:(j - g + 1) * 128],
                            src_b16[:, j * 128:(j + 1) * 128],
                            ident[:],
                            is_transpose=True,
                        )
                    ncopy = min(4, n_kt - g)
                    dst_ap = dst[:, dst_ti0 + g:dst_ti0 + g + ncopy, dst_col:dst_col + 128]
                    src_ap = tp[:, :ncopy * 128].rearrange("p (a f) -> p a f", f=128)
                    if eng_sel % 2 == 0:
                        nc.vector.tensor_copy(dst_ap, src_ap)
                    else:
                        nc.scalar.copy(dst_ap, src_ap)
                    eng_sel += 1

            # ---- load + transpose w_f (4 h-tiles, each [128, 4096])
            for a in range(NHT):
                wchunk = wload.tile([128, NK * 128], BF16, tag="wload")
                nc.gpsimd.dma_start(wchunk[:], d_wf[a * 128:(a + 1) * 128, :])
                transpose_chunk(wchunk, NK, wfT, 0, a * 128, a)

            # ---- phase 1 per batch tile
            for bt in range(NB):
                bs = bt * 128
                achunk = aload.tile([128, NKH * 128], BF16, tag="aload")
                nc.gpsimd.dma_start(achunk[:], d_hx[bs:bs + 128, :])
                transpose_chunk(achunk, NKH, hxT, 0, bs, bt)
                ichunk = aload.tile([128, NKI * 128], BF16, tag="aload")
                nc.gpsimd.dma_start(ichunk[:], d_inp[bs:bs + 128, :])
                transpose_chunk(ichunk, NKI, inputT, 0, bs, bt + 1)
                nc.sync.dma_start(hxcN[:, bt, :], d_hxc[bs:bs + 128, :])

                acc = ps_acc.tile([128, HC], F32, tag="acc")
                for j in range(NK):
                    lhsT = (hxT[:, j, bs:bs + 128] if j < NKH
                            else inputT[:, j - NKH, bs:bs + 128])
                    nc.tensor.matmul(
                        acc[:], lhsT, wfT[:, j, :],
                        start=(j == 0), stop=(j == NK - 1),
                    )
                # fg = 0.5*acc + bfp ; omfg = bfm - 0.5*acc ; fgx = fg * hxc
                fg_t = fgtr.tile([128, HC], BF16, tag="fg")
                nc.vector.scalar_tensor_tensor(
                    fg_t[:], acc[:], 0.5, bfp_bc,
                    mybir.AluOpType.mult, mybir.AluOpType.add)
                nc.vector.scalar_tensor_tensor(
                    omfgN[:, bt, :], acc[:], -0.5, bfm_bc,
                    mybir.AluOpType.mult, mybir.AluOpType.add)
                nc.vector.tensor_mul(fghxN[:, bt, :], fg_t[:], hxcN[:, bt, :])
                # transpose fgx tile -> fg_hxT[:, :, bs:bs+128]
                tp = ps_tp.tile([128, 512], BF16, tag="tp")
                for a in range(NHT):
                    nc.tensor.matmul(
                        tp[:, a * 128:(a + 1) * 128],
                        fghxN[:, bt, a * 128:(a + 1) * 128],
                        ident[:],
                        is_transpose=True,
                    )
                nc.scalar.copy(
                    fg_hxT[:, :, bs:bs + 128],
                    tp[:].rearrange("p (a f) -> p a f", f=128),
                )

            # ---- collective: all-gather fgx^T across the 4-core row group
            nc.sync.dma_start(cc_in.rearrange("(a p) b -> p a b", p=128), fg_hxT[:])
            nc.gpsimd.collective_compute(
                "AllGather",
                mybir.AluOpType.bypass,
                replica_groups=[[0, 1, 2, 3], [4, 5, 6, 7]],
                ins=[cc_in.opt()],
                outs=[cc_out.opt()],
            )

            # ---- load + transpose w_n into the slot wfT occupied
            wnT = wT_pool.tile([128, NK, HC], BF16, tag="wT")
            for a in range(NHT):
                wchunk = wload.tile([128, NK * 128], BF16, tag="wload")
                nc.gpsimd.dma_start(wchunk[:], d_wn[a * 128:(a + 1) * 128, :])
                transpose_chunk(wchunk, NK, wnT, 0, a * 128, a)

            # ---- read back gathered fgx^T (reuses hxT's slot)
            gat = big_pool.tile([128, NKH, BL], BF16, tag="big_hx")
            for j in range(NKH):
                q, rr = j // NHT, (j % NHT) * 128
                nc.sync.dma_start(gat[:, j, :], cc_out[q, rr:rr + 128, :])

            # ---- phase 2 per batch tile
            for bt in range(NB):
                bs = bt * 128
                acc = ps_acc.tile([128, HC], F32, tag="acc")
                for j in range(NK):
                    lhsT = (gat[:, j, bs:bs + 128] if j < NKH
                            else inputT[:, j - NKH, bs:bs + 128])
                    nc.tensor.matmul(
                        acc[:], lhsT, wnT[:, j, :],
                        start=(j == 0), stop=(j == NK - 1),
                    )
                # hy = omfg * (acc + bn) + fgx
                t = scr.tile([128, HC], F32, tag="t")
                nc.vector.tensor_add(t[:], acc[:], bn_bc)
                u = scr.tile([128, HC], F32, tag="u")
                nc.vector.tensor_mul(u[:], omfgN[:, bt, :], t[:])
                o = outp.tile([128, HC], F32, tag="o")
                nc.vector.tensor_add(o[:], u[:], fghxN[:, bt, :])
                nc.sync.dma_start(d_hy[bs:bs + 128, :], o[:])

    nc.finalize()
    return nc


def _get_nc():
    global _NC_CACHE
    if _NC_CACHE is None:
        _NC_CACHE = build()
    return _NC_CACHE


def kernel(input, hx, w_f, b_f, w_n, b_n, **_ignored):
    input = np.ascontiguousarray(np.asarray(input, dtype=np.float32))
    hx = np.ascontiguousarray(np.asarray(hx, dtype=np.float32))
    w_f = np.ascontiguousarray(np.asarray(w_f, dtype=np.float32))
    b_f = np.ascontiguousarray(np.asarray(b_f, dtype=np.float32))
    w_n = np.ascontiguousarray(np.asarray(w_n, dtype=np.float32))
    b_n = np.ascontiguousarray(np.asarray(b_n, dtype=np.float32))

    nc = _get_nc()
    in_maps = []
    for core in range(R * C):
        r, c = core // C, core % C
        in_maps.append({
            "inp": np.ascontiguousarray(input[r * BL:(r + 1) * BL, :]),
            "hx": np.ascontiguousarray(hx[r * BL:(r + 1) * BL, :]),
            "hxc": np.ascontiguousarray(hx[r * BL:(r + 1) * BL, c * HC:(c + 1) * HC]),
            "wf": np.ascontiguousarray(w_f[c * HC:(c + 1) * HC, :]),
            "wn": np.ascontiguousarray(w_n[c * HC:(c + 1) * HC, :]),
            "bf": np.ascontiguousarray(b_f[None, c * HC:(c + 1) * HC]),
            "bn": np.ascontiguousarray(b_n[None, c * HC:(c + 1) * HC]),
        })
    res = run_bass_kernel_spmd(nc, in_maps, list(range(R * C)))
    rows = []
    for r in range(R):
        rows.append(np.concatenate(
            [res.results[r * C + c]["hy"] for c in range(C)], axis=1))
    return np.concatenate(rows, axis=0)


if __name__ == "__main__":
    rng = np.random.default_rng(0)
    inputs = {
        "input": rng.uniform(-1, 1, (B, I)).astype(np.float32),
        "hx": rng.uniform(-1, 1, (B, H)).astype(np.float32),
        "w_f": (rng.standard_normal((H, H + I)) / np.sqrt(H + I)).astype(np.float32),
        "b_f": (rng.standard_normal(H) / np.sqrt(H + I)).astype(np.float32),
        "w_n": (rng.standard_normal((H, H + I)) / np.sqrt(H + I)).astype(np.float32),
        "b_n": (rng.standard_normal(H) / np.sqrt(H + I)).astype(np.float32),
    }
    out = kernel(**inputs)
    x64 = {k: v.astype(np.float64) for k, v in inputs.items()}
    cat = np.concatenate([x64["hx"], x64["input"]], axis=1)
    fg = (cat @ x64["w_f"].T + x64["b_f"] + 1.0) * 0.5
    fgx = fg * x64["hx"]
    ng = np.concatenate([fgx, x64["input"]], axis=1) @ x64["w_n"].T + x64["b_n"]
    exp = (1.0 - fg) * ng + fgx
    err = np.abs(out - exp).max() / np.abs(exp).max()
    print("rel err:", err)


# revision 14
# speedup vs baseline: 1.1065x; 1.1065x over previous
"""FSUMGU cell on 8 Trainium2 NeuronCores.

Math (per reference):
    zf = [hx, x] @ w_f.T + b_f
    fg = (zf + 1) / 2
    fgx = fg * hx
    ng = [fgx, x] @ w_n.T + b_n
    hy = (1 - fg) * ng + fgx

Sharding: 2 batch-halves (r) x 4 hidden-quarters (c); core id = r*4 + c.
Each core computes hy[r-half, c-quarter]. The only cross-core dependency
is ng's contraction over the full hidden dim of fgx, satisfied with one
AllGather of bf16 fgx^T over each 4-core row group.

On-core: activations/weights are PE-transposed into [k, *] bf16 SBUF
tiles so every matmul is out[b,h] += catT[k,b].T @ wT[k,h] with fp32
PSUM accumulation. fp32->bf16 happens inside SWDGE cast-DMAs. PE
transposes are interleaved into the matmul stream in small groups so
the tensor engine never idles long enough for HAM to re-throttle the
clock. Phase 2 accumulates its input-half contraction first so those
matmuls (plus the w_n transposes) hide the AllGather latency.
"""
import sys

sys.path.insert(0, "/opt/trn_rl_repo")

import numpy as np
import concourse.bass as bass
import concourse.tile as tile
from concourse import bacc, mybir, masks
from concourse.bass_utils import run_bass_kernel_spmd

F32 = mybir.dt.float32
BF16 = mybir.dt.bfloat16

B, H, I = 2048, 2048, 2048
R, C = 2, 4
BL = B // R            # 1024 rows of batch per core
HC = H // C            # 512 output features per core
NB = BL // 128         # 8 batch tiles
NKH = H // 128         # 16 k-tiles in the hx / fgx part
NKI = I // 128         # 16 k-tiles in the input part
NK = NKH + NKI         # 32 k-tiles total contraction
NHT = HC // 128        # 4 h-tiles per core slice

_NC_CACHE = None


def build():
    nc = bacc.Bacc(None, target_bir_lowering=False, debug=False)
    d_inp = nc.dram_tensor("inp", [BL, I], F32, kind="ExternalInput").ap()
    d_hx = nc.dram_tensor("hx", [BL, H], F32, kind="ExternalInput").ap()
    d_hxc = nc.dram_tensor("hxc", [BL, HC], F32, kind="ExternalInput").ap()
    d_wf = nc.dram_tensor("wf", [HC, H + I], F32, kind="ExternalInput").ap()
    d_wn = nc.dram_tensor("wn", [HC, H + I], F32, kind="ExternalInput").ap()
    d_bf = nc.dram_tensor("bf", [1, HC], F32, kind="ExternalInput").ap()
    d_bn = nc.dram_tensor("bn", [1, HC], F32, kind="ExternalInput").ap()
    d_hy = nc.dram_tensor("hy", [BL, HC], F32, kind="ExternalOutput").ap()

    with tile.TileContext(nc) as tc:
        with (
            tc.tile_pool(name="const", bufs=1) as const,
            tc.tile_pool(name="wT", bufs=1) as wT_pool,          # wfT then wnT (time-shared)
            tc.tile_pool(name="big", bufs=1) as big_pool,        # hxT then gather (time-shared) + inputT
            tc.tile_pool(name="persist", bufs=1) as persist,
            tc.tile_pool(name="aload", bufs=4) as aload,
            tc.tile_pool(name="wload", bufs=4) as wload,
            tc.tile_pool(name="scr", bufs=2) as scr,
            tc.tile_pool(name="fgtr", bufs=2) as fgtr,
            tc.tile_pool(name="outp", bufs=2) as outp,
            tc.tile_pool(name="dram", bufs=1, space="DRAM") as dram,
            tc.tile_pool(name="ps_acc", bufs=4, space="PSUM") as ps_acc,
            tc.tile_pool(name="ps_tp", bufs=3, space="PSUM") as ps_tp,
            tc.tile_pool(name="ps_b", bufs=1, space="PSUM") as ps_b,
        ):
            ident = const.tile([128, 128], BF16, tag="ident")
            masks.make_identity(nc, ident[:])
            ones = const.tile([1, 128], BF16, tag="ones")
            nc.vector.memset(ones[:], 1.0)

            # ---- bias prep: bfp=(b_f+1)/2, bfm=1-bfp, bn; broadcast to 128 partitions
            bf_row = const.tile([1, HC], F32, tag="bfrow")
            bn_row = const.tile([1, HC], F32, tag="bnrow")
            nc.sync.dma_start(bf_row[:], d_bf[:])
            nc.sync.dma_start(bn_row[:], d_bn[:])
            bfp_row = const.tile([1, HC], F32, tag="bfprow")
            bfm_row = const.tile([1, HC], F32, tag="bfmrow")
            nc.vector.tensor_scalar(bfp_row[:], bf_row[:], 0.5, 0.5,
                                    mybir.AluOpType.mult, mybir.AluOpType.add)
            nc.vector.tensor_scalar(bfm_row[:], bfp_row[:], -1.0, 1.0,
                                    mybir.AluOpType.mult, mybir.AluOpType.add)
            bias_bc = const.tile([128, 3, HC], BF16, tag="biasbc")
            for bi, row in enumerate((bfp_row, bfm_row, bn_row)):
                row16 = const.tile([1, HC], BF16, tag=f"row16_{bi}")
                nc.vector.tensor_copy(row16[:], row[:])
                pb = ps_b.tile([128, HC], F32, tag="pbias")
                nc.tensor.matmul(pb[:], ones[:], row16[:], start=True, stop=True)
                nc.vector.tensor_copy(bias_bc[:, bi, :], pb[:])
            bfp_bc = bias_bc[:, 0, :]
            bfm_bc = bias_bc[:, 1, :]
            bn_bc = bias_bc[:, 2, :]

            # ---- persistent transposed tensors
            hxT = big_pool.tile([128, NKH, BL], BF16, tag="big_hx")      # hx^T  (phase 1)
            inputT = big_pool.tile([128, NKI, BL], BF16, tag="big_inp")  # input^T (both phases)
            wfT = wT_pool.tile([128, NK, HC], BF16, tag="wT")
            fg_hxT = persist.tile([128, NHT, BL], BF16, tag="fghxT")
            omfgN = persist.tile([128, NB, HC], BF16, tag="omfg")
            fghxN = persist.tile([128, NB, HC], BF16, tag="fghx")
            hxcN = persist.tile([128, NB, HC], BF16, tag="hxc")

            # DRAM bounce buffers for the collective
            cc_in = dram.tile([HC, BL], BF16)
            cc_out = dram.tile([C, HC, BL], BF16)

            eng_state = [0]

            def emit_tp_group(src_b16, src_k0, n_kt, dst, dst_ti0, dst_col):
                """PE-transpose n_kt (<=4) [128,128] slices + one batched copy."""
                tp = ps_tp.tile([128, 512], BF16, tag="tp")
                for j in range(n_kt):
                    nc.tensor.matmul(
                        tp[:, j * 128:(j + 1) * 128],
                        src_b16[:, (src_k0 + j) * 128:(src_k0 + j + 1) * 128],
                        ident[:],
                        is_transpose=True,
                    )
                dst_ap = dst[:, dst_ti0:dst_ti0 + n_kt, dst_col:dst_col + 128]
                src_ap = tp[:, :n_kt * 128].rearrange("p (a f) -> p a f", f=128)
                if eng_state[0] % 2 == 0:
                    nc.vector.tensor_copy(dst_ap, src_ap)
                else:
                    nc.scalar.copy(dst_ap, src_ap)
                eng_state[0] += 1

            filler = []   # queued (tag, fn) transpose groups, drained between MM bursts

            def drain(n):
                for _ in range(min(n, len(filler))):
                    filler.pop(0)[1]()

            def drain_until(tag):
                """Emit every queued group with tag <= `tag` (correctness gate)."""
                while filler and filler[0][0] <= tag:
                    filler.pop(0)[1]()

            def queue_act_tiles(bt):
                """Load + queue transposes for b-tile bt's activations."""
                bs = bt * 128
                achunk = aload.tile([128, NKH * 128], BF16, tag="aload")
                nc.gpsimd.dma_start(achunk[:], d_hx[bs:bs + 128, :])
                ichunk = aload.tile([128, NKI * 128], BF16, tag="aload")
                nc.gpsimd.dma_start(ichunk[:], d_inp[bs:bs + 128, :])
                nc.gpsimd.dma_start(hxcN[:, bt, :], d_hxc[bs:bs + 128, :])
                for g in range(0, NKH, 4):
                    filler.append((bt, lambda g=g, t=achunk: emit_tp_group(t, g, 4, hxT, g, bs)))
                for g in range(0, NKI, 4):
                    filler.append((bt, lambda g=g, t=ichunk: emit_tp_group(t, g, 4, inputT, g, bs)))

            # ---- w_f: load k-half-major so early k-tiles are ready first
            for kh in range(2):
                for a in range(NHT):
                    wchunk = wload.tile([128, NKH * 128], BF16, tag="wload")
                    nc.gpsimd.dma_start(
                        wchunk[:], d_wf[a * 128:(a + 1) * 128, kh * 2048:(kh + 1) * 2048])
                    for g in range(0, NKH, 4):
                        emit_tp_group(wchunk, g, 4, wfT, kh * NKH + g, a * 128)

            # prime activations for the first two b-tiles (direct emission)
            queue_act_tiles(0)
            queue_act_tiles(1)
            drain_until(1)

            # ---- phase 1 per batch tile: dense MM stream + interleaved fillers
            for bt in range(NB):
                bs = bt * 128
                if bt + 2 < NB:
                    queue_act_tiles(bt + 2)
                drain_until(bt)  # this tile's operands must be emitted already
                acc = ps_acc.tile([128, HC], F32, tag="acc")
                for j in range(NK):
                    if j % 8 == 4:
                        drain(1)
                    lhsT = (hxT[:, j, bs:bs + 128] if j < NKH
                            else inputT[:, j - NKH, bs:bs + 128])
                    nc.tensor.matmul(
                        acc[:], lhsT, wfT[:, j, :],
                        start=(j == 0), stop=(j == NK - 1),
                    )
                # fg = 0.5*acc + bfp ; omfg = bfm - 0.5*acc ; fgx = fg * hxc
                fg_t = fgtr.tile([128, HC], BF16, tag="fg")
                nc.vector.scalar_tensor_tensor(
                    fg_t[:], acc[:], 0.5, bfp_bc,
                    mybir.AluOpType.mult, mybir.AluOpType.add)
                nc.vector.scalar_tensor_tensor(
                    omfgN[:, bt, :], acc[:], -0.5, bfm_bc,
                    mybir.AluOpType.mult, mybir.AluOpType.add)
                nc.vector.tensor_mul(fghxN[:, bt, :], fg_t[:], hxcN[:, bt, :])
                # transpose fgx tile -> fg_hxT[:, :, bs:bs+128] (small, HAM-safe)
                tp = ps_tp.tile([128, 512], BF16, tag="tp")
                for a in range(NHT):
                    nc.tensor.matmul(
                        tp[:, a * 128:(a + 1) * 128],
                        fghxN[:, bt, a * 128:(a + 1) * 128],
                        ident[:],
                        is_transpose=True,
                    )
                nc.scalar.copy(
                    fg_hxT[:, :, bs:bs + 128],
                    tp[:].rearrange("p (a f) -> p a f", f=128),
                )
                # stream this b-tile's fgx^T columns to the collective input
                nc.sync.dma_start(
                    cc_in.rearrange("(a p) b -> p a b", p=128)[:, :, bs:bs + 128],
                    fg_hxT[:, :, bs:bs + 128])

            # ---- collective: all-gather fgx^T across the 4-core row group
            nc.gpsimd.collective_compute(
                "AllGather",
                mybir.AluOpType.bypass,
                replica_groups=[[0, 1, 2, 3], [4, 5, 6, 7]],
                ins=[cc_in.opt()],
                outs=[cc_out.opt()],
            )

            # ---- w_n: load input-half (k-tiles 16..31) first, transpose all.
            # This dense block (plus phase-2's input-half matmuls) runs during
            # the AllGather, so PE idle time there is already covered.
            wnT = wT_pool.tile([128, NK, HC], BF16, tag="wT")
            for kh in (1, 0):
                for a in range(NHT):
                    wchunk = wload.tile([128, NKH * 128], BF16, tag="wload")
                    nc.gpsimd.dma_start(
                        wchunk[:], d_wn[a * 128:(a + 1) * 128, kh * 2048:(kh + 1) * 2048])
                    for g in range(0, NKH, 4):
                        emit_tp_group(wchunk, g, 4, wnT, kh * NKH + g, a * 128)
            drain(len(filler))  # flush any remaining queued act groups
            assert not filler

            # ---- read back gathered fgx^T (reuses hxT's slot)
            gat = big_pool.tile([128, NKH, BL], BF16, tag="big_hx")
            for j in range(NKH):
                q, rr = j // NHT, (j % NHT) * 128
                nc.sync.dma_start(gat[:, j, :], cc_out[q, rr:rr + 128, :])

            # ---- phase 2 per batch tile: input half first (independent of CC)
            for bt in range(NB):
                bs = bt * 128
                acc = ps_acc.tile([128, HC], F32, tag="acc")
                korder = list(range(NKH, NK)) + list(range(NKH))
                for idx, j in enumerate(korder):
                    lhsT = (gat[:, j, bs:bs + 128] if j < NKH
                            else inputT[:, j - NKH, bs:bs + 128])
                    nc.tensor.matmul(
                        acc[:], lhsT, wnT[:, j, :],
                        start=(idx == 0), stop=(idx == NK - 1),
                    )
                # hy = omfg * (acc + bn) + fgx
                t = scr.tile([128, HC], F32, tag="t")
                nc.vector.tensor_add(t[:], acc[:], bn_bc)
                u = scr.tile([128, HC], F32, tag="u")
                nc.vector.tensor_mul(u[:], omfgN[:, bt, :], t[:])
                o = outp.tile([128, HC], F32, tag="o")
                nc.vector.tensor_add(o[:], u[:], fghxN[:, bt, :])
                nc.sync.dma_start(d_hy[bs:bs + 128, :], o[:])

    nc.finalize()
    return nc


def _get_nc():
    global _NC_CACHE
    if _NC_CACHE is None:
        _NC_CACHE = build()
    return _NC_CACHE


def kernel(input, hx, w_f, b_f, w_n, b_n, **_ignored):
    input = np.ascontiguousarray(np.asarray(input, dtype=np.float32))
    hx = np.ascontiguousarray(np.asarray(hx, dtype=np.float32))
    w_f = np.ascontiguousarray(np.asarray(w_f, dtype=np.float32))
    b_f = np.ascontiguousarray(np.asarray(b_f, dtype=np.float32))
    w_n = np.ascontiguousarray(np.asarray(w_n, dtype=np.float32))
    b_n = np.ascontiguousarray(np.asarray(b_n, dtype=np.float32))

    nc = _get_nc()
    in_maps = []
    for core in range(R * C):
        r, c = core // C, core % C
        in_maps.append({
            "inp": np.ascontiguousarray(input[r * BL:(r + 1) * BL, :]),
            "hx": np.ascontiguousarray(hx[r * BL:(r + 1) * BL, :]),
            "hxc": np.ascontiguousarray(hx[r * BL:(r + 1) * BL, c * HC:(c + 1) * HC]),
            "wf": np.ascontiguousarray(w_f[c * HC:(c + 1) * HC, :]),
            "wn": np.ascontiguousarray(w_n[c * HC:(c + 1) * HC, :]),
            "bf": np.ascontiguousarray(b_f[None, c * HC:(c + 1) * HC]),
            "bn": np.ascontiguousarray(b_n[None, c * HC:(c + 1) * HC]),
        })
    res = run_bass_kernel_spmd(nc, in_maps, list(range(R * C)))
    rows = []
    for r in range(R):
        rows.append(np.concatenate(
            [res.results[r * C + c]["hy"] for c in range(C)], axis=1))
    return np.concatenate(rows, axis=0)


if __name__ == "__main__":
    rng = np.random.default_rng(0)
    inputs = {
        "input": rng.uniform(-1, 1, (B, I)).astype(np.float32),
        "hx": rng.uniform(-1, 1, (B, H)).astype(np.float32),
        "w_f": (rng.standard_normal((H, H + I)) / np.sqrt(H + I)).astype(np.float32),
        "b_f": (rng.standard_normal(H) / np.sqrt(H + I)).astype(np.float32),
        "w_n": (rng.standard_normal((H, H + I)) / np.sqrt(H + I)).astype(np.float32),
        "b_n": (rng.standard_normal(H) / np.sqrt(H + I)).astype(np.float32),
    }
    out = kernel(**inputs)
    x64 = {k: v.astype(np.float64) for k, v in inputs.items()}
    cat = np.concatenate([x64["hx"], x64["input"]], axis=1)
    fg = (cat @ x64["w_f"].T + x64["b_f"] + 1.0) * 0.5
    fgx = fg * x64["hx"]
    ng = np.concatenate([fgx, x64["input"]], axis=1) @ x64["w_n"].T + x64["b_n"]
    exp = (1.0 - fg) * ng + fgx
    err = np.abs(out - exp).max() / np.abs(exp).max()
    print("rel err:", err)


# revision 16
# speedup vs baseline: 1.1525x; 1.0416x over previous
"""FSUMGU cell on 8 Trainium2 NeuronCores.

Math (per reference):
    zf = [hx, x] @ w_f.T + b_f
    fg = (zf + 1) / 2
    fgx = fg * hx
    ng = [fgx, x] @ w_n.T + b_n
    hy = (1 - fg) * ng + fgx

Sharding: 2 batch-halves (r) x 4 hidden-quarters (c); core id = r*4 + c.
Each core computes hy[r-half, c-quarter]. The only cross-core dependency
is ng's contraction over the full hidden dim of fgx, satisfied with one
AllGather of bf16 fgx^T over each 4-core row group.

On-core: activations/weights are PE-transposed into [k, *] bf16 SBUF
tiles so every matmul is out[b,h] += catT[k,b].T @ wT[k,h] with fp32
PSUM accumulation. fp32->bf16 happens inside SWDGE cast-DMAs. PE
transposes are interleaved into the matmul stream in small groups so
the tensor engine never idles long enough for HAM to re-throttle the
clock. Phase 2 accumulates its input-half contraction first so those
matmuls (plus the w_n transposes) hide the AllGather latency.
"""
import sys

sys.path.insert(0, "/opt/trn_rl_repo")

import numpy as np
import concourse.bass as bass
import concourse.tile as tile
from concourse import bacc, mybir, masks
from concourse.bass_utils import run_bass_kernel_spmd

F32 = mybir.dt.float32
BF16 = mybir.dt.bfloat16

B, H, I = 2048, 2048, 2048
R, C = 2, 4
BL = B // R            # 1024 rows of batch per core
HC = H // C            # 512 output features per core
NB = BL // 128         # 8 batch tiles
NKH = H // 128         # 16 k-tiles in the hx / fgx part
NKI = I // 128         # 16 k-tiles in the input part
NK = NKH + NKI         # 32 k-tiles total contraction
NHT = HC // 128        # 4 h-tiles per core slice

_NC_CACHE = None


def build():
    nc = bacc.Bacc(None, target_bir_lowering=False, debug=False)
    d_inp = nc.dram_tensor("inp", [BL, I], F32, kind="ExternalInput").ap()
    d_hx = nc.dram_tensor("hx", [BL, H], F32, kind="ExternalInput").ap()
    d_hxc = nc.dram_tensor("hxc", [BL, HC], F32, kind="ExternalInput").ap()
    d_wf = nc.dram_tensor("wf", [HC, H + I], F32, kind="ExternalInput").ap()
    d_wn = nc.dram_tensor("wn", [HC, H + I], F32, kind="ExternalInput").ap()
    d_bf = nc.dram_tensor("bf", [1, HC], F32, kind="ExternalInput").ap()
    d_bn = nc.dram_tensor("bn", [1, HC], F32, kind="ExternalInput").ap()
    d_hy = nc.dram_tensor("hy", [BL, HC], F32, kind="ExternalOutput").ap()

    with tile.TileContext(nc) as tc:
        with (
            tc.tile_pool(name="const", bufs=1) as const,
            tc.tile_pool(name="wT", bufs=1) as wT_pool,          # wfT then wnT (time-shared)
            tc.tile_pool(name="big", bufs=1) as big_pool,        # hxT then gather (time-shared) + inputT
            tc.tile_pool(name="persist", bufs=1) as persist,
            tc.tile_pool(name="aload", bufs=4) as aload,
            tc.tile_pool(name="wload", bufs=3) as wload,
            tc.tile_pool(name="scr", bufs=2) as scr,
            tc.tile_pool(name="fgtr", bufs=2) as fgtr,
            tc.tile_pool(name="outp", bufs=2) as outp,
            tc.tile_pool(name="dram", bufs=1, space="DRAM") as dram,
            tc.tile_pool(name="ps_acc", bufs=5, space="PSUM") as ps_acc,
            tc.tile_pool(name="ps_tp", bufs=2, space="PSUM") as ps_tp,
            tc.tile_pool(name="ps_b", bufs=1, space="PSUM") as ps_b,
        ):
            ident = const.tile([128, 128], BF16, tag="ident")
            masks.make_identity(nc, ident[:])
            ones = const.tile([1, 128], BF16, tag="ones")
            nc.vector.memset(ones[:], 1.0)

            # ---- persistent transposed tensors
            hxT = big_pool.tile([128, NKH, BL], BF16, tag="big_hx")      # hx^T  (phase 1)
            inputT = big_pool.tile([128, NKI, BL], BF16, tag="big_inp")  # input^T (both phases)
            wfT = wT_pool.tile([128, NK, HC], BF16, tag="wT")
            fg_hxT = persist.tile([128, NHT, BL], BF16, tag="fghxT")
            omfgN = persist.tile([128, NB, HC], BF16, tag="omfg")
            fghxN = persist.tile([128, NB, HC], BF16, tag="fghx")
            hxcN = persist.tile([128, NB, HC], BF16, tag="hxc")

            # DRAM bounce buffers for the two half-batch collectives
            HB = BL // 2
            cc_in1 = dram.tile([HC, HB], BF16)
            cc_in2 = dram.tile([HC, HB], BF16)
            cc_out1 = dram.tile([C, HC, HB], BF16)
            cc_out2 = dram.tile([C, HC, HB], BF16)

            eng_state = [0]

            def emit_tp_group(src_b16, src_k0, n_kt, dst, dst_ti0, dst_col):
                """PE-transpose n_kt (<=4) [128,128] slices + one batched copy."""
                tp = ps_tp.tile([128, 512], BF16, tag="tp")
                for j in range(n_kt):
                    nc.tensor.matmul(
                        tp[:, j * 128:(j + 1) * 128],
                        src_b16[:, (src_k0 + j) * 128:(src_k0 + j + 1) * 128],
                        ident[:],
                        is_transpose=True,
                    )
                dst_ap = dst[:, dst_ti0:dst_ti0 + n_kt, dst_col:dst_col + 128]
                src_ap = tp[:, :n_kt * 128].rearrange("p (a f) -> p a f", f=128)
                if eng_state[0] % 2 == 0:
                    nc.vector.tensor_copy(dst_ap, src_ap)
                else:
                    nc.scalar.copy(dst_ap, src_ap)
                eng_state[0] += 1

            filler = []   # queued (tag, fn) transpose groups, drained between MM bursts

            def drain(n):
                for _ in range(min(n, len(filler))):
                    filler.pop(0)[1]()

            def drain_until(tag):
                """Emit every queued group with tag <= `tag` (correctness gate)."""
                while filler and filler[0][0] <= tag:
                    filler.pop(0)[1]()

            def queue_act_tiles(bt):
                """Load + queue transposes for b-tile bt's activations."""
                bs = bt * 128
                achunk = aload.tile([128, NKH * 128], BF16, tag="aload")
                nc.gpsimd.dma_start(achunk[:], d_hx[bs:bs + 128, :])
                ichunk = aload.tile([128, NKI * 128], BF16, tag="aload")
                nc.gpsimd.dma_start(ichunk[:], d_inp[bs:bs + 128, :])
                nc.gpsimd.dma_start(hxcN[:, bt, :], d_hxc[bs:bs + 128, :])
                for g in range(0, NKH, 4):
                    filler.append((bt, lambda g=g, t=achunk: emit_tp_group(t, g, 4, hxT, g, bs)))
                for g in range(0, NKI, 4):
                    filler.append((bt, lambda g=g, t=ichunk: emit_tp_group(t, g, 4, inputT, g, bs)))

            # ---- w_f k-half 0 + first activations: minimal deps for first matmul
            def load_wf_half(kh, dst):
                for a in range(NHT):
                    wchunk = wload.tile([128, NKH * 128], BF16, tag="wload")
                    nc.gpsimd.dma_start(
                        wchunk[:], d_wf[a * 128:(a + 1) * 128, kh * 2048:(kh + 1) * 2048])
                    for g in range(0, NKH, 4):
                        emit_tp_group(wchunk, g, 4, dst, kh * NKH + g, a * 128)

            load_wf_half(0, wfT)
            queue_act_tiles(0)
            queue_act_tiles(1)
            drain_until(1)
            load_wf_half(1, wfT)

            # ---- bias prep: bfp=(b_f+1)/2, bfm=1-bfp, bn; broadcast to 128 partitions
            bf_row = const.tile([1, HC], F32, tag="bfrow")
            bn_row = const.tile([1, HC], F32, tag="bnrow")
            nc.sync.dma_start(bf_row[:], d_bf[:])
            nc.sync.dma_start(bn_row[:], d_bn[:])
            bfp_row = const.tile([1, HC], F32, tag="bfprow")
            bfm_row = const.tile([1, HC], F32, tag="bfmrow")
            nc.vector.tensor_scalar(bfp_row[:], bf_row[:], 0.5, 0.5,
                                    mybir.AluOpType.mult, mybir.AluOpType.add)
            nc.vector.tensor_scalar(bfm_row[:], bfp_row[:], -1.0, 1.0,
                                    mybir.AluOpType.mult, mybir.AluOpType.add)
            bias_bc = const.tile([128, 3, HC], BF16, tag="biasbc")
            for bi, row in enumerate((bfp_row, bfm_row, bn_row)):
                row16 = const.tile([1, HC], BF16, tag=f"row16_{bi}")
                nc.vector.tensor_copy(row16[:], row[:])
                pb = ps_b.tile([128, HC], F32, tag="pbias")
                nc.tensor.matmul(pb[:], ones[:], row16[:], start=True, stop=True)
                nc.vector.tensor_copy(bias_bc[:, bi, :], pb[:])
            bfp_bc = bias_bc[:, 0, :]
            bfm_bc = bias_bc[:, 1, :]
            bn_bc = bias_bc[:, 2, :]

            # ---- phase 1 per batch tile: dense MM stream + interleaved fillers
            for bt in range(NB):
                bs = bt * 128
                if bt + 2 < NB:
                    queue_act_tiles(bt + 2)
                drain_until(bt)  # this tile's operands must be emitted already
                acc = ps_acc.tile([128, HC], F32, tag="acc")
                for j in range(NK):
                    if j % 8 == 4:
                        drain(1)
                    lhsT = (hxT[:, j, bs:bs + 128] if j < NKH
                            else inputT[:, j - NKH, bs:bs + 128])
                    nc.tensor.matmul(
                        acc[:], lhsT, wfT[:, j, :],
                        start=(j == 0), stop=(j == NK - 1),
                    )
                # fg = 0.5*acc + bfp ; omfg = bfm - 0.5*acc ; fgx = fg * hxc
                fg_t = fgtr.tile([128, HC], BF16, tag="fg")
                nc.vector.scalar_tensor_tensor(
                    fg_t[:], acc[:], 0.5, bfp_bc,
                    mybir.AluOpType.mult, mybir.AluOpType.add)
                nc.vector.scalar_tensor_tensor(
                    omfgN[:, bt, :], acc[:], -0.5, bfm_bc,
                    mybir.AluOpType.mult, mybir.AluOpType.add)
                nc.vector.tensor_mul(fghxN[:, bt, :], fg_t[:], hxcN[:, bt, :])
                # transpose fgx tile -> fg_hxT[:, :, bs:bs+128] (small, HAM-safe)
                tp = ps_tp.tile([128, 512], BF16, tag="tp")
                for a in range(NHT):
                    nc.tensor.matmul(
                        tp[:, a * 128:(a + 1) * 128],
                        fghxN[:, bt, a * 128:(a + 1) * 128],
                        ident[:],
                        is_transpose=True,
                    )
                nc.scalar.copy(
                    fg_hxT[:, :, bs:bs + 128],
                    tp[:].rearrange("p (a f) -> p a f", f=128),
                )
                # stream this b-tile's fgx^T columns to the collective input
                cc_in_half = cc_in1 if bt < NB // 2 else cc_in2
                hb = bs if bt < NB // 2 else bs - HB
                nc.sync.dma_start(
                    cc_in_half.rearrange("(a p) b -> p a b", p=128)[:, :, hb:hb + 128],
                    fg_hxT[:, :, bs:bs + 128])
                if bt == NB // 2 - 1:
                    # first-half all-gather rides under the rest of phase 1
                    nc.gpsimd.collective_compute(
                        "AllGather",
                        mybir.AluOpType.bypass,
                        replica_groups=[[0, 1, 2, 3], [4, 5, 6, 7]],
                        ins=[cc_in1.opt()],
                        outs=[cc_out1.opt()],
                    )

            # ---- second-half all-gather
            nc.gpsimd.collective_compute(
                "AllGather",
                mybir.AluOpType.bypass,
                replica_groups=[[0, 1, 2, 3], [4, 5, 6, 7]],
                ins=[cc_in2.opt()],
                outs=[cc_out2.opt()],
            )

            # ---- w_n: load input-half (k-tiles 16..31) first, transpose all.
            # This dense block (plus phase-2's input-half matmuls) runs during
            # the AllGather, so PE idle time there is already covered.
            wnT = wT_pool.tile([128, NK, HC], BF16, tag="wT")
            for kh in (1, 0):
                for a in range(NHT):
                    wchunk = wload.tile([128, NKH * 128], BF16, tag="wload")
                    nc.gpsimd.dma_start(
                        wchunk[:], d_wn[a * 128:(a + 1) * 128, kh * 2048:(kh + 1) * 2048])
                    for g in range(0, NKH, 4):
                        emit_tp_group(wchunk, g, 4, wnT, kh * NKH + g, a * 128)
            drain(len(filler))  # flush any remaining queued act groups
            assert not filler

            # ---- read back gathered fgx^T (reuses hxT's slot)
            gat = big_pool.tile([128, NKH, BL], BF16, tag="big_hx")
            for j in range(NKH):
                q, rr = j // NHT, (j % NHT) * 128
                nc.sync.dma_start(gat[:, j, :HB], cc_out1[q, rr:rr + 128, :])
                nc.sync.dma_start(gat[:, j, HB:], cc_out2[q, rr:rr + 128, :])

            # ---- phase 2 per batch tile: input half first (independent of CC)
            for bt in range(NB):
                bs = bt * 128
                acc = ps_acc.tile([128, HC], F32, tag="acc")
                korder = list(range(NKH, NK)) + list(range(NKH))
                for idx, j in enumerate(korder):
                    lhsT = (gat[:, j, bs:bs + 128] if j < NKH
                            else inputT[:, j - NKH, bs:bs + 128])
                    nc.tensor.matmul(
                        acc[:], lhsT, wnT[:, j, :],
                        start=(idx == 0), stop=(idx == NK - 1),
                    )
                # hy = omfg * (acc + bn) + fgx
                t = scr.tile([128, HC], F32, tag="t")
                nc.vector.tensor_add(t[:], acc[:], bn_bc)
                u = scr.tile([128, HC], F32, tag="u")
                nc.vector.tensor_mul(u[:], omfgN[:, bt, :], t[:])
                o = outp.tile([128, HC], F32, tag="o")
                nc.vector.tensor_add(o[:], u[:], fghxN[:, bt, :])
                nc.sync.dma_start(d_hy[bs:bs + 128, :], o[:])

    nc.finalize()
    return nc


def _get_nc():
    global _NC_CACHE
    if _NC_CACHE is None:
        _NC_CACHE = build()
    return _NC_CACHE


def kernel(input, hx, w_f, b_f, w_n, b_n, **_ignored):
    input = np.ascontiguousarray(np.asarray(input, dtype=np.float32))
    hx = np.ascontiguousarray(np.asarray(hx, dtype=np.float32))
    w_f = np.ascontiguousarray(np.asarray(w_f, dtype=np.float32))
    b_f = np.ascontiguousarray(np.asarray(b_f, dtype=np.float32))
    w_n = np.ascontiguousarray(np.asarray(w_n, dtype=np.float32))
    b_n = np.ascontiguousarray(np.asarray(b_n, dtype=np.float32))

    nc = _get_nc()
    in_maps = []
    for core in range(R * C):
        r, c = core // C, core % C
        in_maps.append({
            "inp": np.ascontiguousarray(input[r * BL:(r + 1) * BL, :]),
            "hx": np.ascontiguousarray(hx[r * BL:(r + 1) * BL, :]),
            "hxc": np.ascontiguousarray(hx[r * BL:(r + 1) * BL, c * HC:(c + 1) * HC]),
            "wf": np.ascontiguousarray(w_f[c * HC:(c + 1) * HC, :]),
            "wn": np.ascontiguousarray(w_n[c * HC:(c + 1) * HC, :]),
            "bf": np.ascontiguousarray(b_f[None, c * HC:(c + 1) * HC]),
            "bn": np.ascontiguousarray(b_n[None, c * HC:(c + 1) * HC]),
        })
    res = run_bass_kernel_spmd(nc, in_maps, list(range(R * C)))
    rows = []
    for r in range(R):
        rows.append(np.concatenate(
            [res.results[r * C + c]["hy"] for c in range(C)], axis=1))
    return np.concatenate(rows, axis=0)


if __name__ == "__main__":
    rng = np.random.default_rng(0)
    inputs = {
        "input": rng.uniform(-1, 1, (B, I)).astype(np.float32),
        "hx": rng.uniform(-1, 1, (B, H)).astype(np.float32),
        "w_f": (rng.standard_normal((H, H + I)) / np.sqrt(H + I)).astype(np.float32),
        "b_f": (rng.standard_normal(H) / np.sqrt(H + I)).astype(np.float32),
        "w_n": (rng.standard_normal((H, H + I)) / np.sqrt(H + I)).astype(np.float32),
        "b_n": (rng.standard_normal(H) / np.sqrt(H + I)).astype(np.float32),
    }
    out = kernel(**inputs)
    x64 = {k: v.astype(np.float64) for k, v in inputs.items()}
    cat = np.concatenate([x64["hx"], x64["input"]], axis=1)
    fg = (cat @ x64["w_f"].T + x64["b_f"] + 1.0) * 0.5
    fgx = fg * x64["hx"]
    ng = np.concatenate([fgx, x64["input"]], axis=1) @ x64["w_n"].T + x64["b_n"]
    exp = (1.0 - fg) * ng + fgx
    err = np.abs(out - exp).max() / np.abs(exp).max()
    print("rel err:", err)
